# revision 26
# baseline (speedup 1.0000x reference)
"""Trainium2 Bass kernel for T5-style cross-attention, sharded over 8 NeuronCores.

Sharding: tensor-parallel over heads (16 heads -> 2 per core). Each core
computes Q/K/V projections for its 2 heads (full batch), flash-style
attention with multiplicative exp(position_bias), and a partial output
projection against its row-slice of Wo. The host sums the 8 partial
outputs (the unshard step for a row-sharded Wo).

v3: everything bf16 on the wire (PSUM accumulation fp32). The additive
position bias is applied as exp(bias) on the Vector engine after the exp
(attn = exp(S) * exp(bias)), so no PE bias matmuls. Flash sweeps run in
(q-half, head) order with a [65, 1024] PSUM accumulator per (sweep,
batch); the softmax denominator comes from a ones-column appended to V.
Projections are emitted interleaved into sweep 0's flash loop and the
output projection into sweeps 2/3, so PE work overlaps the ACT-bound
flash pipeline. Inputs load as a handful of large rearranged DMAs
(dma_start issue overhead is ~1.8us each on the SP sequencer).
"""

import sys

try:
    import concourse.bass as bass
except ImportError:
    sys.path.insert(0, "/opt/trn_rl_repo")
    import concourse.bass as bass

import numpy as np
import ml_dtypes
_ml_bf16 = ml_dtypes.bfloat16

import concourse.mybir as mybir
from concourse import bacc
from concourse.tile import TileContext
from concourse.bass_utils import run_bass_kernel_spmd

F32 = mybir.dt.float32
F16 = mybir.dt.float16
BF16 = mybir.dt.bfloat16

# Problem sizes (hardcoded per spec)
B, NQ, NKV = 4, 2048, 2048
D_MODEL, N_HEADS, D_K = 1024, 16, 64
N_CORES = 8
HPC = N_HEADS // N_CORES          # heads per core = 2
DH = HPC * D_K                    # 128 partition rows of per-core head dims

QH = 1024                         # q half (flash sweep / u accumulator span)
N_QH = NQ // QH                   # 2
KT = 128                          # k tile (partition dim of S^T)
N_KT = NKV // KT                  # 16
MW = 512                          # matmul moving width (fp32 PSUM bank limit)
KG = 4                            # k tiles per exp(bias) DMA group


def build_kernel(b=B, nq=NQ, nkv=NKV, d_model=D_MODEL):
    nc = bacc.Bacc("TRN2", target_bir_lowering=False, debug=False,
                   num_devices=N_CORES)

    xT = nc.dram_tensor("xT", [b, d_model, nq], F16, kind="ExternalInput")
    encT = nc.dram_tensor("encT", [b, d_model, nkv], F16, kind="ExternalInput")
    expbT = nc.dram_tensor("expbT", [HPC, nkv, nq], BF16, kind="ExternalInput")
    wq = nc.dram_tensor("wq", [d_model, DH], F16, kind="ExternalInput")
    wk = nc.dram_tensor("wk", [d_model, DH], F16, kind="ExternalInput")
    wv = nc.dram_tensor("wv", [d_model, DH], F16, kind="ExternalInput")
    wo = nc.dram_tensor("wo", [DH, d_model], BF16, kind="ExternalInput")
    identb = nc.dram_tensor("identb", [128, 128], BF16, kind="ExternalInput")
    out = nc.dram_tensor("out", [b, nq, d_model], BF16, kind="ExternalOutput")

    n_m = d_model // 128          # model-dim tiles (8)

    with TileContext(nc) as tc:
        with (
            tc.tile_pool(name="cst", bufs=1) as cst,
            tc.tile_pool(name="wpool", bufs=1) as wpool,
            tc.tile_pool(name="qkv", bufs=1) as qkv,
            tc.tile_pool(name="stage", bufs=2) as stage,
            tc.tile_pool(name="vtstage", bufs=2) as vtstage,
            tc.tile_pool(name="ebp", bufs=4) as ebp,
            tc.tile_pool(name="sattn", bufs=4) as sattn,
            tc.tile_pool(name="sattnb", bufs=4) as sattnb,
            tc.tile_pool(name="unorm", bufs=3) as unorm,
            tc.tile_pool(name="snorm", bufs=3) as snorm,
            tc.tile_pool(name="sout", bufs=4) as sout,
            tc.tile_pool(name="psbig", bufs=2, space="PSUM") as psbig,
            tc.tile_pool(name="psu", bufs=1, space="PSUM") as psu,
            tc.tile_pool(name="psa", bufs=2, space="PSUM") as psa,
        ):
            # ---- constants & weights (one DMA each, rearranged) ----
            ident_bf = cst.tile([128, 128], BF16, tag="identbf")
            nc.sync.dma_start(out=ident_bf, in_=identb[:, :])

            w_sb = {}
            for nm, t in (("q", wq), ("k", wk), ("v", wv)):
                w_sb[nm] = wpool.tile([128, n_m * DH], F16, tag=f"w{nm}",
                                      name=f"w{nm}")
                nc.sync.dma_start(
                    out=w_sb[nm].rearrange("p (m d) -> p m d", m=n_m),
                    in_=t.rearrange("(m p) d -> p m d", p=128))
            wo_sb = wpool.tile([128, d_model], BF16, tag="wo")
            nc.sync.dma_start(out=wo_sb, in_=wo[:, :])

            # ---- persistent per-batch activations ----
            qT_sb = [qkv.tile([128, nq], F16, tag=f"qT{bi}", name=f"qT{bi}")
                     for bi in range(b)]
            kT_sb = [qkv.tile([128, nkv], F16, tag=f"kT{bi}", name=f"kT{bi}")
                     for bi in range(b)]
            ctx_t = [qkv.tile([128, nq], BF16, tag=f"ctx{bi}", name=f"ctx{bi}")
                     for bi in range(b)]
            # pair-packed Vones tiles: [h0 V(64) | ones | h1 V(64) | ones]
            vones = {}
            for bi in range(b):
                for kt in range(N_KT):
                    t = qkv.tile([128, HPC * (D_K + 1)], BF16,
                                 tag=f"v_{bi}_{kt}", name=f"v_{bi}_{kt}")
                    vones[(bi, kt)] = t
                    nc.gpsimd.memset(
                        t.rearrange("p (h c) -> p h c",
                                    h=HPC)[:, :, D_K:D_K + 1], 1.0)

            # ---- phase A emission units (projections for one batch) ----
            def a_units(bi):
                st = {}

                def load(src, key, half):
                    def f():
                        t = stage.tile([128, n_m * QH], F16, tag="stage",
                                       name=f"st_{key}_{bi}_{half}")
                        nc.sync.dma_start(
                            out=t.rearrange("p (m q) -> p m q", m=n_m),
                            in_=src[bi, :, half * QH:(half + 1) * QH]
                            .rearrange("(m p) q -> p m q", p=128))
                        st[(key, half)] = t
                    return f

                def proj_kv(w):
                    def f():
                        half, off = w // 2, (w % 2) * MW
                        et = st[("e", half)]
                        k_ps = psa.tile([128, MW], F32, tag="a",
                                        name=f"kps_{bi}_{w}")
                        for m in range(n_m):
                            nc.tensor.matmul(
                                k_ps, w_sb["k"][:, m * DH:(m + 1) * DH],
                                et[:, m * QH + off:m * QH + off + MW],
                                start=(m == 0), stop=(m == n_m - 1))
                        nc.scalar.copy(
                            kT_sb[bi][:, w * MW:(w + 1) * MW], k_ps)
                        v_ps = psa.tile([128, MW], F32, tag="a",
                                        name=f"vps_{bi}_{w}")
                        for m in range(n_m):
                            nc.tensor.matmul(
                                v_ps, w_sb["v"][:, m * DH:(m + 1) * DH],
                                et[:, m * QH + off:m * QH + off + MW],
                                start=(m == 0), stop=(m == n_m - 1))
                        vt_win = vtstage.tile([128, MW], BF16, tag="vtw")
                        nc.scalar.copy(vt_win, v_ps)
                        vt_ps = psa.tile([128, MW], BF16, tag="a",
                                         name=f"vtp_{bi}_{w}")
                        for s in range(MW // 128):
                            nc.tensor.transpose(
                                vt_ps[:, s * 128:(s + 1) * 128],
                                vt_win[:, s * 128:(s + 1) * 128], ident_bf)
                        for s in range(MW // 128):
                            kt = (w * MW + s * 128) // KT
                            nc.vector.tensor_copy(
                                vones[(bi, kt)].rearrange(
                                    "p (h c) -> p h c", h=HPC)[:, :, 0:D_K],
                                vt_ps[:, s * 128:(s + 1) * 128]
                                .rearrange("p (h c) -> p h c", h=HPC))
                    return f

                def proj_q(w):
                    def f():
                        half, off = w // 2, (w % 2) * MW
                        xt = st[("x", half)]
                        q_ps = psa.tile([128, MW], F32, tag="a",
                                        name=f"qps_{bi}_{w}")
                        for m in range(n_m):
                            nc.tensor.matmul(
                                q_ps, w_sb["q"][:, m * DH:(m + 1) * DH],
                                xt[:, m * QH + off:m * QH + off + MW],
                                start=(m == 0), stop=(m == n_m - 1))
                        nc.scalar.copy(
                            qT_sb[bi][:, w * MW:(w + 1) * MW], q_ps)
                    return f

                return [load(encT, "e", 0), load(encT, "e", 1),
                        proj_kv(0), proj_kv(1),
                        load(xT, "x", 0), proj_kv(2), proj_kv(3),
                        load(xT, "x", 1),
                        proj_q(0), proj_q(1), proj_q(2), proj_q(3)]

            # ---- output projection chunk units (one 128-q chunk each) ----
            def wo_chunk(bi, qs, on_act=False):
                def f():
                    o_sb = sout.tile([128, d_model], BF16, tag="out")
                    for e in range(d_model // MW):
                        o_ps = psa.tile([128, MW], F32, tag="a",
                                        name=f"ops_{bi}_{qs}_{e}")
                        nc.tensor.matmul(
                            o_ps, ctx_t[bi][:, qs * 128:(qs + 1) * 128],
                            wo_sb[:, e * MW:(e + 1) * MW],
                            start=True, stop=True)
                        dst = o_sb[:, e * MW:(e + 1) * MW]
                        if on_act:
                            nc.scalar.copy(dst, o_ps)
                        else:
                            nc.vector.tensor_copy(dst, o_ps)
                    nc.sync.dma_start(
                        out=out[bi, qs * 128:(qs + 1) * 128, :], in_=o_sb)
                return f

            wo_queue = []

            # ---- flash sweeps ----
            sweeps = [(0, 0), (0, 1), (1, 0), (1, 1)]   # (qh, h)

            # emit A(b0) up front
            for u in a_units(0):
                u()

            for si, (qh, h) in enumerate(sweeps):
                hp = h * D_K
                q0 = qh * QH
                # exp(bias) cache for this sweep: 4 big DMAs
                eb = []
                for g in range(N_KT // KG):
                    t = ebp.tile([128, KG * QH], BF16, tag="eb",
                                 name=f"eb_{si}_{g}")
                    nc.sync.dma_start(
                        out=t.rearrange("p (k q) -> p k q", k=KG),
                        in_=expbT[h, g * KG * KT:(g + 1) * KG * KT,
                                  q0:q0 + QH]
                        .rearrange("(k p) q -> p k q", p=KT))
                    eb.append(t)

                for bi in range(b):
                    # filler units interleaved into this (sweep, batch):
                    # sweep 0 carries the next batch's projections (every
                    # slot); later sweeps drain the Wo queue (1 per 4 slots
                    # to keep DVE under the ACT-bound sweep rate)
                    if si == 0 and bi < b - 1:
                        fillers = a_units(bi + 1)
                        rate = 1
                    else:
                        fillers = wo_queue
                        rate = 4

                    u = psu.tile([D_K + 1, QH], F32, tag="u",
                                 name=f"u_{si}_{bi}")
                    for kt in range(N_KT):
                        s_ps = psbig.tile([128, QH], F32, tag="big",
                                          name=f"sg_{si}_{bi}_{kt}")
                        for s in range(QH // MW):
                            nc.tensor.matmul(
                                s_ps[:, s * MW:(s + 1) * MW],
                                kT_sb[bi][hp:hp + D_K,
                                          kt * KT:(kt + 1) * KT],
                                qT_sb[bi][hp:hp + D_K,
                                          q0 + s * MW:q0 + (s + 1) * MW],
                                start=True, stop=True)
                        attn = sattn.tile([128, QH], BF16, tag="attn")
                        nc.scalar.activation(
                            attn, s_ps, mybir.ActivationFunctionType.Exp)
                        attnb = sattnb.tile([128, QH], BF16, tag="attnb")
                        nc.vector.tensor_mul(
                            attnb, attn,
                            eb[kt // KG][:, (kt % KG) * QH:
                                         (kt % KG + 1) * QH])
                        o = h * (D_K + 1)
                        for s in range(QH // MW):
                            nc.tensor.matmul(
                                u[:, s * MW:(s + 1) * MW],
                                vones[(bi, kt)][:, o:o + D_K + 1],
                                attnb[:, s * MW:(s + 1) * MW],
                                start=(kt == 0), stop=(kt == N_KT - 1),
                                skip_group_check=True)
                        if fillers and kt % rate == rate - 1:
                            f = fillers.pop(0)
                            if callable(f):
                                f()
                            else:
                                wo_chunk(*f)()
                    # drain u out of PSUM quickly (frees the psu buf), then
                    # normalize off the critical path (mul on idle Pool)
                    u_sb = unorm.tile([D_K + 1, QH], F32, tag="u")
                    nc.vector.tensor_copy(u_sb, u)
                    recip = snorm.tile([1, QH], F32, tag="recip")
                    nc.vector.reciprocal(recip, u_sb[D_K:D_K + 1, :])
                    rb = snorm.tile([D_K, QH], F32, tag="rb")
                    nc.gpsimd.partition_broadcast(rb, recip)
                    with nc.allow_low_precision(reason="bf16 ctx for PE"):
                        nc.gpsimd.tensor_mul(
                            ctx_t[bi][hp:hp + D_K, q0:q0 + QH],
                            u_sb[0:D_K, :], rb)
                    # ctx halves complete after the h=1 sweeps -> queue Wo
                    if si == 1:
                        wo_queue.extend((bi, qs) for qs in range(QH // 128))
                    elif si == 3:
                        wo_queue.extend(
                            (bi, qs) for qs in range(QH // 128, nq // 128))

            # tail: drain remaining Wo chunks, copies alternating ACT/DVE
            for i, (tbi, tqs) in enumerate(wo_queue):
                wo_chunk(tbi, tqs, on_act=(i % 2 == 0))()
    nc.compile()
    return nc


_NC_CACHE = {}


def _get_nc():
    if "nc" not in _NC_CACHE:
        _NC_CACHE["nc"] = build_kernel()
    return _NC_CACHE["nc"]


def prepare_in_maps(x, encoding, position_bias, Wq, Wk, Wv, Wo):
    x = np.asarray(x, np.float32)
    encoding = np.asarray(encoding, np.float32)
    position_bias = np.asarray(position_bias, np.float32)
    Wq = np.asarray(Wq, np.float32)
    Wk = np.asarray(Wk, np.float32)
    Wv = np.asarray(Wv, np.float32)
    Wo = np.asarray(Wo, np.float32)

    xT = np.ascontiguousarray(x.transpose(0, 2, 1)).astype(np.float16)
    encT = np.ascontiguousarray(encoding.transpose(0, 2, 1)).astype(np.float16)
    expb = np.exp(position_bias[0])          # [16, NQ, NKV] fp32
    identb = np.eye(128, dtype=_ml_bf16)

    in_maps = []
    for c in range(N_CORES):
        h0 = c * HPC
        in_maps.append({
            "xT": xT,
            "encT": encT,
            "expbT": np.ascontiguousarray(
                expb[h0:h0 + HPC].transpose(0, 2, 1)).astype(_ml_bf16),
            "wq": np.ascontiguousarray(
                Wq[:, h0 * D_K:(h0 + HPC) * D_K]).astype(np.float16),
            "wk": np.ascontiguousarray(
                Wk[:, h0 * D_K:(h0 + HPC) * D_K]).astype(np.float16),
            "wv": np.ascontiguousarray(
                Wv[:, h0 * D_K:(h0 + HPC) * D_K]).astype(np.float16),
            "wo": np.ascontiguousarray(
                Wo[h0 * D_K:(h0 + HPC) * D_K, :]).astype(_ml_bf16),
            "identb": identb,
        })
    return in_maps


def kernel(x, encoding, position_bias, Wq, Wk, Wv, Wo):
    in_maps = prepare_in_maps(x, encoding, position_bias, Wq, Wk, Wv, Wo)
    nc = _get_nc()
    res = run_bass_kernel_spmd(nc, in_maps, list(range(N_CORES)))
    acc = res.results[0]["out"].astype(np.float32)
    for c in range(1, N_CORES):
        acc = acc + res.results[c]["out"].astype(np.float32)
    return acc


# revision 28
# speedup vs baseline: 1.2280x; 1.2280x over previous
"""Trainium2 Bass kernel for T5-style cross-attention, sharded over 8 NeuronCores.

Sharding: tensor-parallel over heads (16 heads -> 2 per core). Each core
computes Q/K/V projections for its 2 heads (full batch), flash-style
attention with multiplicative exp(position_bias), and a partial output
projection against its row-slice of Wo. The host sums the 8 partial
outputs (the unshard step for a row-sharded Wo).

v3: everything bf16 on the wire (PSUM accumulation fp32). The additive
position bias is applied as exp(bias) on the Vector engine after the exp
(attn = exp(S) * exp(bias)), so no PE bias matmuls. Flash sweeps run in
(q-half, head) order with a [65, 1024] PSUM accumulator per (sweep,
batch); the softmax denominator comes from a ones-column appended to V.
Projections are emitted interleaved into sweep 0's flash loop and the
output projection into sweeps 2/3, so PE work overlaps the ACT-bound
flash pipeline. Inputs load as a handful of large rearranged DMAs
(dma_start issue overhead is ~1.8us each on the SP sequencer).
"""

import sys

try:
    import concourse.bass as bass
except ImportError:
    sys.path.insert(0, "/opt/trn_rl_repo")
    import concourse.bass as bass

import numpy as np
import ml_dtypes
_ml_bf16 = ml_dtypes.bfloat16

import concourse.mybir as mybir
from concourse import bacc
from concourse.tile import TileContext
from concourse.bass_utils import run_bass_kernel_spmd

F32 = mybir.dt.float32
F16 = mybir.dt.float16
BF16 = mybir.dt.bfloat16

# Problem sizes (hardcoded per spec)
B, NQ, NKV = 4, 2048, 2048
D_MODEL, N_HEADS, D_K = 1024, 16, 64
N_CORES = 8
HPC = N_HEADS // N_CORES          # heads per core = 2
DH = HPC * D_K                    # 128 partition rows of per-core head dims

QH = 1024                         # q half (flash sweep / u accumulator span)
N_QH = NQ // QH                   # 2
KT = 128                          # k tile (partition dim of S^T)
N_KT = NKV // KT                  # 16
MW = 512                          # matmul moving width (fp32 PSUM bank limit)
KG = 4                            # k tiles per exp(bias) DMA group


def build_kernel(b=B, nq=NQ, nkv=NKV, d_model=D_MODEL):
    nc = bacc.Bacc("TRN2", target_bir_lowering=False, debug=False,
                   num_devices=N_CORES)

    xT = nc.dram_tensor("xT", [b, d_model, nq], F16, kind="ExternalInput")
    encT = nc.dram_tensor("encT", [b, d_model, nkv], F16, kind="ExternalInput")
    expbT = nc.dram_tensor("expbT", [HPC, nkv, nq], BF16, kind="ExternalInput")
    wq = nc.dram_tensor("wq", [d_model, DH], F16, kind="ExternalInput")
    wk = nc.dram_tensor("wk", [d_model, DH], F16, kind="ExternalInput")
    wv = nc.dram_tensor("wv", [d_model, DH], F16, kind="ExternalInput")
    wo = nc.dram_tensor("wo", [DH, d_model], BF16, kind="ExternalInput")
    identb = nc.dram_tensor("identb", [128, 128], BF16, kind="ExternalInput")
    out = nc.dram_tensor("out", [b, nq, d_model], BF16, kind="ExternalOutput")

    n_m = d_model // 128          # model-dim tiles (8)

    with TileContext(nc) as tc:
        with (
            tc.tile_pool(name="cst", bufs=1) as cst,
            tc.tile_pool(name="wpool", bufs=1) as wpool,
            tc.tile_pool(name="qkv", bufs=1) as qkv,
            tc.tile_pool(name="stage", bufs=2) as stage,
            tc.tile_pool(name="vtstage", bufs=2) as vtstage,
            tc.tile_pool(name="ebp", bufs=4) as ebp,
            tc.tile_pool(name="sattn", bufs=4) as sattn,
            tc.tile_pool(name="sattnb", bufs=4) as sattnb,
            tc.tile_pool(name="unorm", bufs=3) as unorm,
            tc.tile_pool(name="snorm", bufs=3) as snorm,
            tc.tile_pool(name="sout", bufs=4) as sout,
            tc.tile_pool(name="psbig", bufs=2, space="PSUM") as psbig,
            tc.tile_pool(name="psu", bufs=1, space="PSUM") as psu,
            tc.tile_pool(name="psa", bufs=2, space="PSUM") as psa,
        ):
            # ---- constants & weights (one DMA each, rearranged) ----
            ident_bf = cst.tile([128, 128], BF16, tag="identbf")
            nc.sync.dma_start(out=ident_bf, in_=identb[:, :])

            w_sb = {}
            for nm, t in (("q", wq), ("k", wk), ("v", wv)):
                w_sb[nm] = wpool.tile([128, n_m * DH], F16, tag=f"w{nm}",
                                      name=f"w{nm}")
                nc.sync.dma_start(
                    out=w_sb[nm].rearrange("p (m d) -> p m d", m=n_m),
                    in_=t.rearrange("(m p) d -> p m d", p=128))
            wo_sb = wpool.tile([128, d_model], BF16, tag="wo")
            nc.sync.dma_start(out=wo_sb, in_=wo[:, :])

            # ---- persistent per-batch activations ----
            qT_sb = [qkv.tile([128, nq], F16, tag=f"qT{bi}", name=f"qT{bi}")
                     for bi in range(b)]
            kT_sb = [qkv.tile([128, nkv], F16, tag=f"kT{bi}", name=f"kT{bi}")
                     for bi in range(b)]
            ctx_t = [qkv.tile([128, nq], BF16, tag=f"ctx{bi}", name=f"ctx{bi}")
                     for bi in range(b)]
            # pair-packed Vones tiles: [h0 V(64) | ones | h1 V(64) | ones]
            vones = {}
            for bi in range(b):
                for kt in range(N_KT):
                    t = qkv.tile([128, HPC * (D_K + 1)], BF16,
                                 tag=f"v_{bi}_{kt}", name=f"v_{bi}_{kt}")
                    vones[(bi, kt)] = t
                    nc.gpsimd.memset(
                        t.rearrange("p (h c) -> p h c",
                                    h=HPC)[:, :, D_K:D_K + 1], 1.0)

            # ---- phase A emission units (projections for one batch) ----
            def a_units(bi):
                st = {}

                def load(src, key, half):
                    def f():
                        t = stage.tile([128, n_m * QH], F16, tag="stage",
                                       name=f"st_{key}_{bi}_{half}")
                        nc.sync.dma_start(
                            out=t.rearrange("p (m q) -> p m q", m=n_m),
                            in_=src[bi, :, half * QH:(half + 1) * QH]
                            .rearrange("(m p) q -> p m q", p=128))
                        st[(key, half)] = t
                    return f

                def proj_kv(w):
                    def f():
                        half, off = w // 2, (w % 2) * MW
                        et = st[("e", half)]
                        k_ps = psa.tile([128, MW], F32, tag="a",
                                        name=f"kps_{bi}_{w}")
                        for m in range(n_m):
                            nc.tensor.matmul(
                                k_ps, w_sb["k"][:, m * DH:(m + 1) * DH],
                                et[:, m * QH + off:m * QH + off + MW],
                                start=(m == 0), stop=(m == n_m - 1))
                        nc.scalar.copy(
                            kT_sb[bi][:, w * MW:(w + 1) * MW], k_ps)
                        v_ps = psa.tile([128, MW], F32, tag="a",
                                        name=f"vps_{bi}_{w}")
                        for m in range(n_m):
                            nc.tensor.matmul(
                                v_ps, w_sb["v"][:, m * DH:(m + 1) * DH],
                                et[:, m * QH + off:m * QH + off + MW],
                                start=(m == 0), stop=(m == n_m - 1))
                        vt_win = vtstage.tile([128, MW], BF16, tag="vtw")
                        nc.scalar.copy(vt_win, v_ps)
                        vt_ps = psa.tile([128, MW], BF16, tag="a",
                                         name=f"vtp_{bi}_{w}")
                        for s in range(MW // 128):
                            nc.tensor.transpose(
                                vt_ps[:, s * 128:(s + 1) * 128],
                                vt_win[:, s * 128:(s + 1) * 128], ident_bf)
                        for s in range(MW // 128):
                            kt = (w * MW + s * 128) // KT
                            nc.vector.tensor_copy(
                                vones[(bi, kt)].rearrange(
                                    "p (h c) -> p h c", h=HPC)[:, :, 0:D_K],
                                vt_ps[:, s * 128:(s + 1) * 128]
                                .rearrange("p (h c) -> p h c", h=HPC))
                    return f

                def proj_q(w):
                    def f():
                        half, off = w // 2, (w % 2) * MW
                        xt = st[("x", half)]
                        q_ps = psa.tile([128, MW], F32, tag="a",
                                        name=f"qps_{bi}_{w}")
                        for m in range(n_m):
                            nc.tensor.matmul(
                                q_ps, w_sb["q"][:, m * DH:(m + 1) * DH],
                                xt[:, m * QH + off:m * QH + off + MW],
                                start=(m == 0), stop=(m == n_m - 1))
                        nc.scalar.copy(
                            qT_sb[bi][:, w * MW:(w + 1) * MW], q_ps)
                    return f

                return [load(encT, "e", 0), load(encT, "e", 1),
                        proj_kv(0), proj_kv(1),
                        load(xT, "x", 0), proj_kv(2), proj_kv(3),
                        load(xT, "x", 1),
                        proj_q(0), proj_q(1), proj_q(2), proj_q(3)]

            # ---- output projection chunk units (one 128-q chunk each) ----
            def wo_chunk(bi, qs, on_act=False):
                def f():
                    o_sb = sout.tile([128, d_model], BF16, tag="out")
                    for e in range(d_model // MW):
                        o_ps = psa.tile([128, MW], F32, tag="a",
                                        name=f"ops_{bi}_{qs}_{e}")
                        nc.tensor.matmul(
                            o_ps, ctx_t[bi][:, qs * 128:(qs + 1) * 128],
                            wo_sb[:, e * MW:(e + 1) * MW],
                            start=True, stop=True)
                        dst = o_sb[:, e * MW:(e + 1) * MW]
                        if on_act:
                            nc.scalar.copy(dst, o_ps)
                        else:
                            nc.vector.tensor_copy(dst, o_ps)
                    nc.sync.dma_start(
                        out=out[bi, qs * 128:(qs + 1) * 128, :], in_=o_sb)
                return f

            wo_queue = []

            # ---- flash sweeps ----
            sweeps = [(0, 0), (0, 1), (1, 0), (1, 1)]   # (qh, h)

            # emit A(b0) up front
            for u in a_units(0):
                u()

            for si, (qh, h) in enumerate(sweeps):
                hp = h * D_K
                q0 = qh * QH
                # exp(bias) cache for this sweep: 4 big DMAs
                eb = []
                for g in range(N_KT // KG):
                    t = ebp.tile([128, KG * QH], BF16, tag="eb",
                                 name=f"eb_{si}_{g}")
                    nc.sync.dma_start(
                        out=t.rearrange("p (k q) -> p k q", k=KG),
                        in_=expbT[h, g * KG * KT:(g + 1) * KG * KT,
                                  q0:q0 + QH]
                        .rearrange("(k p) q -> p k q", p=KT))
                    eb.append(t)

                for bi in range(b):
                    # filler units interleaved into this (sweep, batch):
                    # sweep 0 carries the next batch's projections (every
                    # slot); later sweeps drain the Wo queue (1 per 4 slots
                    # to keep DVE under the ACT-bound sweep rate)
                    if si == 0 and bi < b - 1:
                        fillers = a_units(bi + 1)
                        rate = 1
                    else:
                        fillers = wo_queue
                        rate = 4

                    u = psu.tile([D_K + 1, QH], F32, tag="u",
                                 name=f"u_{si}_{bi}")
                    for kt in range(N_KT):
                        s_ps = psbig.tile([128, QH], F32, tag="big",
                                          name=f"sg_{si}_{bi}_{kt}")
                        for s in range(QH // MW):
                            nc.tensor.matmul(
                                s_ps[:, s * MW:(s + 1) * MW],
                                kT_sb[bi][hp:hp + D_K,
                                          kt * KT:(kt + 1) * KT],
                                qT_sb[bi][hp:hp + D_K,
                                          q0 + s * MW:q0 + (s + 1) * MW],
                                start=True, stop=True)
                        attn = sattn.tile([128, QH], BF16, tag="attn")
                        nc.scalar.activation(
                            attn, s_ps, mybir.ActivationFunctionType.Exp)
                        attnb = sattnb.tile([128, QH], BF16, tag="attnb")
                        nc.vector.tensor_mul(
                            attnb, attn,
                            eb[kt // KG][:, (kt % KG) * QH:
                                         (kt % KG + 1) * QH])
                        o = h * (D_K + 1)
                        for s in range(QH // MW):
                            nc.tensor.matmul(
                                u[:, s * MW:(s + 1) * MW],
                                vones[(bi, kt)][:, o:o + D_K + 1],
                                attnb[:, s * MW:(s + 1) * MW],
                                start=(kt == 0), stop=(kt == N_KT - 1),
                                skip_group_check=True)
                        if fillers and kt % rate == rate - 1:
                            f = fillers.pop(0)
                            if callable(f):
                                f()
                            else:
                                wo_chunk(*f)()
                    # drain u out of PSUM quickly (frees the psu buf), then
                    # normalize off the critical path (mul on idle Pool)
                    u_sb = unorm.tile([D_K + 1, QH], F32, tag="u")
                    nc.vector.tensor_copy(u_sb, u)
                    recip = snorm.tile([1, QH], F32, tag="recip")
                    nc.vector.reciprocal(recip, u_sb[D_K:D_K + 1, :])
                    rb = snorm.tile([D_K, QH], F32, tag="rb")
                    nc.gpsimd.partition_broadcast(rb, recip)
                    with nc.allow_low_precision(reason="bf16 ctx for PE"):
                        nc.gpsimd.tensor_mul(
                            ctx_t[bi][hp:hp + D_K, q0:q0 + QH],
                            u_sb[0:D_K, :], rb)
                    # ctx halves complete after the h=1 sweeps -> queue Wo
                    if si == 1:
                        wo_queue.extend((bi, qs) for qs in range(QH // 128))
                    elif si == 3:
                        wo_queue.extend(
                            (bi, qs) for qs in range(QH // 128, nq // 128))

            # tail: drain remaining Wo chunks, copies alternating ACT/DVE
            for i, (tbi, tqs) in enumerate(wo_queue):
                wo_chunk(tbi, tqs, on_act=(i % 2 == 0))()
    nc.compile()
    return nc


_NC_CACHE = {}


def _get_nc():
    if "nc" not in _NC_CACHE:
        _NC_CACHE["nc"] = build_kernel()
    return _NC_CACHE["nc"]


def prepare_in_maps(x, encoding, position_bias, Wq, Wk, Wv, Wo):
    x = np.asarray(x, np.float32)
    encoding = np.asarray(encoding, np.float32)
    position_bias = np.asarray(position_bias, np.float32)
    Wq = np.asarray(Wq, np.float32)
    Wk = np.asarray(Wk, np.float32)
    Wv = np.asarray(Wv, np.float32)
    Wo = np.asarray(Wo, np.float32)

    xT = np.ascontiguousarray(x.transpose(0, 2, 1)).astype(np.float16)
    encT = np.ascontiguousarray(encoding.transpose(0, 2, 1)).astype(np.float16)
    expb = np.exp(position_bias[0])          # [16, NQ, NKV] fp32
    identb = np.eye(128, dtype=_ml_bf16)

    in_maps = []
    for c in range(N_CORES):
        h0 = c * HPC
        in_maps.append({
            "xT": xT,
            "encT": encT,
            "expbT": np.ascontiguousarray(
                expb[h0:h0 + HPC].transpose(0, 2, 1)).astype(_ml_bf16),
            "wq": np.ascontiguousarray(
                Wq[:, h0 * D_K:(h0 + HPC) * D_K]).astype(np.float16),
            "wk": np.ascontiguousarray(
                Wk[:, h0 * D_K:(h0 + HPC) * D_K]).astype(np.float16),
            "wv": np.ascontiguousarray(
                Wv[:, h0 * D_K:(h0 + HPC) * D_K]).astype(np.float16),
            "wo": np.ascontiguousarray(
                Wo[h0 * D_K:(h0 + HPC) * D_K, :]).astype(_ml_bf16),
            "identb": identb,
        })
    return in_maps


def kernel(x, encoding, position_bias, Wq, Wk, Wv, Wo):
    in_maps = prepare_in_maps(x, encoding, position_bias, Wq, Wk, Wv, Wo)
    nc = _get_nc()
    res = run_bass_kernel_spmd(nc, in_maps, list(range(N_CORES)))
    acc = res.results[0]["out"].astype(np.float32)
    for c in range(1, N_CORES):
        acc = acc + res.results[c]["out"].astype(np.float32)
    return acc


# revision 35
# speedup vs baseline: 2.5131x; 2.0466x over previous
"""Trainium2 Bass kernel for T5-style cross-attention, sharded over 8 NeuronCores.

Sharding: tensor-parallel over heads (16 heads -> 2 per core). Each core
computes Q/K/V projections for its 2 heads (full batch), flash-style
attention with multiplicative exp(position_bias), and a partial output
projection against its row-slice of Wo. The host sums the 8 partial
outputs (the unshard step for a row-sharded Wo).

v3: everything bf16 on the wire (PSUM accumulation fp32). The additive
position bias is applied as exp(bias) on the Vector engine after the exp
(attn = exp(S) * exp(bias)), so no PE bias matmuls. Flash sweeps run in
(q-half, head) order with a [65, 1024] PSUM accumulator per (sweep,
batch); the softmax denominator comes from a ones-column appended to V.
Projections are emitted interleaved into sweep 0's flash loop and the
output projection into sweeps 2/3, so PE work overlaps the ACT-bound
flash pipeline. Inputs load as a handful of large rearranged DMAs
(dma_start issue overhead is ~1.8us each on the SP sequencer).
"""

import sys

try:
    import concourse.bass as bass
except ImportError:
    sys.path.insert(0, "/opt/trn_rl_repo")
    import concourse.bass as bass

import numpy as np
import ml_dtypes
_ml_bf16 = ml_dtypes.bfloat16

import concourse.mybir as mybir
from concourse import bacc
from concourse.tile import TileContext
from concourse.bass_utils import run_bass_kernel_spmd

F32 = mybir.dt.float32
F16 = mybir.dt.float16
BF16 = mybir.dt.bfloat16

# Problem sizes (hardcoded per spec)
B, NQ, NKV = 4, 2048, 2048
D_MODEL, N_HEADS, D_K = 1024, 16, 64
N_CORES = 8
HPC = N_HEADS // N_CORES          # heads per core = 2
DH = HPC * D_K                    # 128 partition rows of per-core head dims

QH = 1024                         # q half (flash sweep / u accumulator span)
N_QH = NQ // QH                   # 2
KT = 128                          # k tile (partition dim of S^T)
N_KT = NKV // KT                  # 16
MW = 512                          # matmul moving width (fp32 PSUM bank limit)
KG = 4                            # k tiles per exp(bias) DMA group


def build_kernel(b=B, nq=NQ, nkv=NKV, d_model=D_MODEL):
    nc = bacc.Bacc("TRN2", target_bir_lowering=False, debug=False,
                   num_devices=N_CORES)

    xT = nc.dram_tensor("xT", [b, d_model, nq], F16, kind="ExternalInput")
    encT = nc.dram_tensor("encT", [b, d_model, nkv], F16, kind="ExternalInput")
    expbT = nc.dram_tensor("expbT", [HPC, nkv, nq], BF16, kind="ExternalInput")
    wq = nc.dram_tensor("wq", [d_model, DH], F16, kind="ExternalInput")
    wk = nc.dram_tensor("wk", [d_model, DH], F16, kind="ExternalInput")
    wv = nc.dram_tensor("wv", [d_model, DH], F16, kind="ExternalInput")
    wo = nc.dram_tensor("wo", [DH, d_model], BF16, kind="ExternalInput")
    identb = nc.dram_tensor("identb", [128, 128], BF16, kind="ExternalInput")
    out = nc.dram_tensor("out", [b, nq, d_model], BF16, kind="ExternalOutput")

    n_m = d_model // 128          # model-dim tiles (8)

    with TileContext(nc) as tc:
        with (
            tc.tile_pool(name="cst", bufs=1) as cst,
            tc.tile_pool(name="wpool", bufs=1) as wpool,
            tc.tile_pool(name="qkv", bufs=1) as qkv,
            tc.tile_pool(name="stage", bufs=2) as stage,
            tc.tile_pool(name="vtstage", bufs=2) as vtstage,
            tc.tile_pool(name="ebp", bufs=4) as ebp,
            tc.tile_pool(name="sattn", bufs=4) as sattn,
            tc.tile_pool(name="sattnb", bufs=4) as sattnb,
            tc.tile_pool(name="unorm", bufs=3) as unorm,
            tc.tile_pool(name="snorm", bufs=3) as snorm,
            tc.tile_pool(name="sout", bufs=4) as sout,
            tc.tile_pool(name="psbig", bufs=2, space="PSUM") as psbig,
            tc.tile_pool(name="psu", bufs=1, space="PSUM") as psu,
            tc.tile_pool(name="psa", bufs=2, space="PSUM") as psa,
        ):
            # ---- constants & weights (one DMA each, rearranged) ----
            ident_bf = cst.tile([128, 128], BF16, tag="identbf")
            nc.sync.dma_start(out=ident_bf, in_=identb[:, :])

            w_sb = {}
            for nm, t in (("q", wq), ("k", wk), ("v", wv)):
                w_sb[nm] = wpool.tile([128, n_m * DH], F16, tag=f"w{nm}",
                                      name=f"w{nm}")
                nc.sync.dma_start(
                    out=w_sb[nm].rearrange("p (m d) -> p m d", m=n_m),
                    in_=t.rearrange("(m p) d -> p m d", p=128))
            wo_sb = wpool.tile([128, d_model], BF16, tag="wo")
            nc.sync.dma_start(out=wo_sb, in_=wo[:, :])

            # ---- persistent per-batch activations ----
            qT_sb = [qkv.tile([128, nq], F16, tag=f"qT{bi}", name=f"qT{bi}")
                     for bi in range(b)]
            kT_sb = [qkv.tile([128, nkv], F16, tag=f"kT{bi}", name=f"kT{bi}")
                     for bi in range(b)]
            ctx_t = [qkv.tile([128, nq], BF16, tag=f"ctx{bi}", name=f"ctx{bi}")
                     for bi in range(b)]
            # pair-packed Vones tiles: [h0 V(64) | ones | h1 V(64) | ones]
            vones = {}
            for bi in range(b):
                for kt in range(N_KT):
                    t = qkv.tile([128, HPC * (D_K + 1)], BF16,
                                 tag=f"v_{bi}_{kt}", name=f"v_{bi}_{kt}")
                    vones[(bi, kt)] = t
                    nc.gpsimd.memset(
                        t.rearrange("p (h c) -> p h c",
                                    h=HPC)[:, :, D_K:D_K + 1], 1.0)

            # ---- phase A emission units (projections for one batch) ----
            def a_units(bi):
                st = {}

                def load(src, key, half):
                    def f():
                        t = stage.tile([128, n_m * QH], F16, tag="stage",
                                       name=f"st_{key}_{bi}_{half}")
                        nc.sync.dma_start(
                            out=t.rearrange("p (m q) -> p m q", m=n_m),
                            in_=src[bi, :, half * QH:(half + 1) * QH]
                            .rearrange("(m p) q -> p m q", p=128))
                        st[(key, half)] = t
                    return f

                def proj_kv(w):
                    def f():
                        half, off = w // 2, (w % 2) * MW
                        et = st[("e", half)]
                        k_ps = psa.tile([128, MW], F32, tag="a",
                                        name=f"kps_{bi}_{w}")
                        for m in range(n_m):
                            nc.tensor.matmul(
                                k_ps, w_sb["k"][:, m * DH:(m + 1) * DH],
                                et[:, m * QH + off:m * QH + off + MW],
                                start=(m == 0), stop=(m == n_m - 1))
                        nc.scalar.copy(
                            kT_sb[bi][:, w * MW:(w + 1) * MW], k_ps)
                        v_ps = psa.tile([128, MW], F32, tag="a",
                                        name=f"vps_{bi}_{w}")
                        for m in range(n_m):
                            nc.tensor.matmul(
                                v_ps, w_sb["v"][:, m * DH:(m + 1) * DH],
                                et[:, m * QH + off:m * QH + off + MW],
                                start=(m == 0), stop=(m == n_m - 1))
                        vt_win = vtstage.tile([128, MW], BF16, tag="vtw")
                        nc.scalar.copy(vt_win, v_ps)
                        vt_ps = psa.tile([128, MW], BF16, tag="a",
                                         name=f"vtp_{bi}_{w}")
                        for s in range(MW // 128):
                            nc.tensor.transpose(
                                vt_ps[:, s * 128:(s + 1) * 128],
                                vt_win[:, s * 128:(s + 1) * 128], ident_bf)
                        for s in range(MW // 128):
                            kt = (w * MW + s * 128) // KT
                            nc.vector.tensor_copy(
                                vones[(bi, kt)].rearrange(
                                    "p (h c) -> p h c", h=HPC)[:, :, 0:D_K],
                                vt_ps[:, s * 128:(s + 1) * 128]
                                .rearrange("p (h c) -> p h c", h=HPC))
                    return f

                def proj_q(w):
                    def f():
                        half, off = w // 2, (w % 2) * MW
                        xt = st[("x", half)]
                        q_ps = psa.tile([128, MW], F32, tag="a",
                                        name=f"qps_{bi}_{w}")
                        for m in range(n_m):
                            nc.tensor.matmul(
                                q_ps, w_sb["q"][:, m * DH:(m + 1) * DH],
                                xt[:, m * QH + off:m * QH + off + MW],
                                start=(m == 0), stop=(m == n_m - 1))
                        nc.scalar.copy(
                            qT_sb[bi][:, w * MW:(w + 1) * MW], q_ps)
                    return f

                return [load(encT, "e", 0), load(encT, "e", 1),
                        proj_kv(0), proj_kv(1),
                        load(xT, "x", 0), proj_kv(2), proj_kv(3),
                        load(xT, "x", 1),
                        proj_q(0), proj_q(1), proj_q(2), proj_q(3)]

            # ---- output projection chunk units (one 128-q chunk each) ----
            def wo_chunk(bi, qs, on_act=False):
                def f():
                    o_sb = sout.tile([128, d_model], BF16, tag="out")
                    for e in range(d_model // MW):
                        o_ps = psa.tile([128, MW], F32, tag="a",
                                        name=f"ops_{bi}_{qs}_{e}")
                        nc.tensor.matmul(
                            o_ps, ctx_t[bi][:, qs * 128:(qs + 1) * 128],
                            wo_sb[:, e * MW:(e + 1) * MW],
                            start=True, stop=True)
                        dst = o_sb[:, e * MW:(e + 1) * MW]
                        if on_act:
                            nc.scalar.copy(dst, o_ps)
                        else:
                            nc.vector.tensor_copy(dst, o_ps)
                    nc.sync.dma_start(
                        out=out[bi, qs * 128:(qs + 1) * 128, :], in_=o_sb)
                return f

            wo_queue = []

            # ---- flash sweeps ----
            sweeps = [(0, 0), (0, 1), (1, 0), (1, 1)]   # (qh, h)

            # emit A(b0) up front
            for u in a_units(0):
                u()

            for si, (qh, h) in enumerate(sweeps):
                hp = h * D_K
                q0 = qh * QH
                # exp(bias) cache for this sweep: 4 big DMAs
                eb = []
                for g in range(N_KT // KG):
                    t = ebp.tile([128, KG * QH], BF16, tag="eb",
                                 name=f"eb_{si}_{g}")
                    nc.sync.dma_start(
                        out=t.rearrange("p (k q) -> p k q", k=KG),
                        in_=expbT[h, g * KG * KT:(g + 1) * KG * KT,
                                  q0:q0 + QH]
                        .rearrange("(k p) q -> p k q", p=KT))
                    eb.append(t)

                for bi in range(b):
                    # filler units interleaved into this (sweep, batch):
                    # sweep 0 carries the next batch's projections (every
                    # slot); later sweeps drain the Wo queue (1 per 4 slots
                    # to keep DVE under the ACT-bound sweep rate)
                    if si == 0 and bi < b - 1:
                        fillers = a_units(bi + 1)
                        rate = 1
                    else:
                        fillers = wo_queue
                        rate = 4

                    u = psu.tile([D_K + 1, QH], F32, tag="u",
                                 name=f"u_{si}_{bi}")
                    for kt in range(N_KT):
                        s_ps = psbig.tile([128, QH], F32, tag="big",
                                          name=f"sg_{si}_{bi}_{kt}")
                        for s in range(QH // MW):
                            nc.tensor.matmul(
                                s_ps[:, s * MW:(s + 1) * MW],
                                kT_sb[bi][hp:hp + D_K,
                                          kt * KT:(kt + 1) * KT],
                                qT_sb[bi][hp:hp + D_K,
                                          q0 + s * MW:q0 + (s + 1) * MW],
                                start=True, stop=True)
                        attn = sattn.tile([128, QH], BF16, tag="attn")
                        nc.scalar.activation(
                            attn, s_ps, mybir.ActivationFunctionType.Exp)
                        attnb = sattnb.tile([128, QH], BF16, tag="attnb")
                        nc.vector.tensor_mul(
                            attnb, attn,
                            eb[kt // KG][:, (kt % KG) * QH:
                                         (kt % KG + 1) * QH])
                        o = h * (D_K + 1)
                        for s in range(QH // MW):
                            nc.tensor.matmul(
                                u[:, s * MW:(s + 1) * MW],
                                vones[(bi, kt)][:, o:o + D_K + 1],
                                attnb[:, s * MW:(s + 1) * MW],
                                start=(kt == 0), stop=(kt == N_KT - 1),
                                skip_group_check=True)
                        if fillers and kt % rate == rate - 1:
                            f = fillers.pop(0)
                            if callable(f):
                                f()
                            else:
                                wo_chunk(*f)()
                    # drain u out of PSUM quickly (frees the psu buf), then
                    # normalize off the critical path (mul on idle Pool)
                    u_sb = unorm.tile([D_K + 1, QH], F32, tag="u")
                    nc.vector.tensor_copy(u_sb, u)
                    recip = snorm.tile([1, QH], F32, tag="recip")
                    nc.vector.reciprocal(recip, u_sb[D_K:D_K + 1, :])
                    rb = snorm.tile([D_K, QH], F32, tag="rb")
                    nc.gpsimd.partition_broadcast(rb, recip)
                    with nc.allow_low_precision(reason="bf16 ctx for PE"):
                        nc.gpsimd.tensor_mul(
                            ctx_t[bi][hp:hp + D_K, q0:q0 + QH],
                            u_sb[0:D_K, :], rb)
                    # ctx halves complete after the h=1 sweeps -> queue Wo
                    if si == 1:
                        wo_queue.extend((bi, qs) for qs in range(QH // 128))
                    elif si == 3:
                        wo_queue.extend(
                            (bi, qs) for qs in range(QH // 128, nq // 128))

            # tail: drain remaining Wo chunks, copies alternating ACT/DVE
            for i, (tbi, tqs) in enumerate(wo_queue):
                wo_chunk(tbi, tqs, on_act=(i % 2 == 0))()
    nc.compile()
    return nc


_NC_CACHE = {}


def _get_nc():
    if "nc" not in _NC_CACHE:
        _NC_CACHE["nc"] = build_kernel()
    return _NC_CACHE["nc"]


def prepare_in_maps(x, encoding, position_bias, Wq, Wk, Wv, Wo):
    x = np.asarray(x, np.float32)
    encoding = np.asarray(encoding, np.float32)
    position_bias = np.asarray(position_bias, np.float32)
    Wq = np.asarray(Wq, np.float32)
    Wk = np.asarray(Wk, np.float32)
    Wv = np.asarray(Wv, np.float32)
    Wo = np.asarray(Wo, np.float32)

    xT = np.ascontiguousarray(x.transpose(0, 2, 1)).astype(np.float16)
    encT = np.ascontiguousarray(encoding.transpose(0, 2, 1)).astype(np.float16)
    expb = np.exp(position_bias[0])          # [16, NQ, NKV] fp32
    identb = np.eye(128, dtype=_ml_bf16)

    in_maps = []
    for c in range(N_CORES):
        h0 = c * HPC
        in_maps.append({
            "xT": xT,
            "encT": encT,
            "expbT": np.ascontiguousarray(
                expb[h0:h0 + HPC].transpose(0, 2, 1)).astype(_ml_bf16),
            "wq": np.ascontiguousarray(
                Wq[:, h0 * D_K:(h0 + HPC) * D_K]).astype(np.float16),
            "wk": np.ascontiguousarray(
                Wk[:, h0 * D_K:(h0 + HPC) * D_K]).astype(np.float16),
            "wv": np.ascontiguousarray(
                Wv[:, h0 * D_K:(h0 + HPC) * D_K]).astype(np.float16),
            "wo": np.ascontiguousarray(
                Wo[h0 * D_K:(h0 + HPC) * D_K, :]).astype(_ml_bf16),
            "identb": identb,
        })
    return in_maps


def kernel(x, encoding, position_bias, Wq, Wk, Wv, Wo):
    in_maps = prepare_in_maps(x, encoding, position_bias, Wq, Wk, Wv, Wo)
    nc = _get_nc()
    res = run_bass_kernel_spmd(nc, in_maps, list(range(N_CORES)))
    acc = res.results[0]["out"].astype(np.float32)
    for c in range(1, N_CORES):
        acc = acc + res.results[c]["out"].astype(np.float32)
    return acc
